# revision 6
# baseline (speedup 1.0000x reference)
"""Trainium2 Bass kernel for CompleteW2MLSupConLoss.

Strategy (8 NeuronCores, SPMD):
  * Host sorts rows by label (stable) and hands every core the full sorted
    feature/label arrays ROTATED so that core c's 1024 anchor rows sit at
    positions [0, 1024).  One identical program runs on all cores; only the
    data differs.  The scalar loss is permutation-invariant, so no unpermute
    is needed -- each core returns two partial sums which the host combines.
  * Sorting makes the positive-pair mask block diagonal: for anchor row-tile
    t (128 rows) all positives live in columns [128t-pad, 128t+128+pad) where
    pad = max_class_count - 1.  The positive-side work (weights, masked sums)
    therefore runs on a narrow window instead of the full 8192 columns.
  * Dense per-tile work is only: 2 accumulating fp32 matmuls (PE), one ACT
    Exp pass with fused row-sum (softmax denominator), and one custom DVE op
      out = (relu(in0*c0 + c1) + c2) * in1,  accum_out = rowsum(out)
    which computes the W2ML negative weight times exp in a single pass.
  * The self-similarity diagonal is excluded exactly: the diagonal 128-col
    segment of the Exp tile is zeroed on the diagonal (multiply by 1-eye with
    fused row-sum) and s_ii is extracted exactly via an eye-masked reduce.

Math (row i, sums over j != i, T = temperature):
  e_ij   = exp((s_ij - 1)/T)          (shift by 1 ~ rowmax; cancels exactly)
  denom  = sum_j e_ij
  wp     = 1 + relu(0.5 - s)          (positive hard-mining weight)
  wn     = 1 + relu((s - 0.3)/0.7)    (negative hard-mining weight)
  A      = sum_{pos j} wp             PS = sum_{pos j} wp*s
  possum = (PS - A)/T - log(denom)*A
  E      = sum_j wn*e - sum_{pos j} wn*e
  negsum = E / denom
  out0   = sum_i possum_i / max(pos_cnt_i, 1)
  out1   = sum_i negsum_i / max(neg_cnt_i, 1)
  loss   = -out0/B + 0.3 * out1/B
"""

import numpy as np
from contextlib import ExitStack

# ---- problem constants (hardcoded per contest contract) --------------------
B_FULL = 8192
D_FEAT = 256
N_CORES = 8
TEMPERATURE = 0.07
THR_POS = 0.5
THR_NEG = 0.3
NEG_LOSS_W = 0.3
CT = 512  # columns per PSUM tile (fp32 moving-operand max)
PT = 128  # partition tile

_prog_cache: dict = {}
LAST_RESULTS = None  # BassKernelResults of the most recent HW run (for test.py)


# ---- custom DVE op ---------------------------------------------------------
def _w2ml_op():
    """(relu(in0*c0 + c1) + c2) * in1 with fused add-reduction.

    Used with (c0=1/0.7, c1=-0.3/0.7, c2=1) for the dense negative pass
    (in0 = sim from PSUM, in1 = exp tile) and with (c0=-1, c1=0.5, c2=1) for
    the windowed positive pass (in1 = positive mask).
    """
    import concourse.dve_ops as dve_ops
    from concourse.dve_spec import Spec, Src0, Src1, C0, C1, C2, Zero, relu, lower, _has_src1
    from concourse.dve_uop import DveOpSpec

    name = "W2ML_WMUL_ANT"
    for op in dve_ops.OPS:
        if op.name == name:
            return op

    def _ref(in0, in1, c0, c1, c2):
        b = ((np.maximum(in0.astype(np.float32) * c0 + c1, 0.0) + c2) * in1).astype(
            np.float32
        )
        return b, b.reshape(b.shape[0], -1).sum(axis=-1, keepdims=True)

    from operator import add

    spec = Spec(body=(relu(Src0 * C0 + C1) + C2) * Src1, accum=add,
                accum_init=Zero, reference=_ref)
    shas = {}
    for ver in ("v3", "v4"):
        try:
            uops = lower(spec, ver=ver)
            shas[ver] = DveOpSpec(name=name, opcode=None, uops=uops,
                                  rd1_en=_has_src1(spec)).sha(ver)
        except Exception:
            pass
    op = dve_ops.DveOp(name, spec, subdim=False, uops_sha=shas)
    row = max(dve_ops._SUB_OPCODE_FOR_NAME.values()) + 1
    assert row < 0x20
    dve_ops.OPS.append(op)
    dve_ops.CUSTOM_DVE_SPECS[name] = spec
    dve_ops._SUB_OPCODE_FOR_NAME[name] = row
    return op


# ---- window geometry (host side) ------------------------------------------
def _window_pieces(t, pad, b_cols):
    """Column pieces [(ct, lo, hi)] of window [128t-pad, 128t+128+pad) mod B."""
    wlo = PT * t - pad
    whi = PT * t + PT + pad
    if whi - wlo >= b_cols:
        segs = [(0, b_cols)]
    elif wlo < 0:
        segs = [(b_cols + wlo, b_cols), (0, whi)]
    elif whi > b_cols:
        segs = [(wlo, b_cols), (0, whi - b_cols)]
    else:
        segs = [(wlo, whi)]
    pieces = []
    for s0, s1 in segs:
        ct0, ct1 = s0 // CT, (s1 - 1) // CT
        for ct in range(ct0, ct1 + 1):
            lo = max(s0, ct * CT) - ct * CT
            hi = min(s1, (ct + 1) * CT) - ct * CT
            if hi > lo:
                pieces.append((ct, lo, hi))
    return pieces


# ---- program builder -------------------------------------------------------
def _build(b_cols, r_rows, pad):
    """Build+compile the per-core Bass program. r_rows = anchor rows per core."""
    import concourse.bass as bass
    import concourse.mybir as mybir
    import concourse.tile as tile
    from concourse import bacc

    op = _w2ml_op()
    f32 = mybir.dt.float32
    AF = mybir.ActivationFunctionType
    ALU = mybir.AluOpType
    AX = mybir.AxisListType

    KB = D_FEAT // PT          # 2 contraction blocks
    NT_F = b_cols // PT        # feature row tiles (64)
    RT = r_rows // PT          # anchor row tiles per core (8)
    NCT = b_cols // CT         # 16 column tiles
    invT = 1.0 / TEMPERATURE

    all_pieces = [_window_pieces(t, pad, b_cols) for t in range(RT)]
    npmax = max(len(p) for p in all_pieces)
    wmax = min(CT, PT + 2 * pad)

    nc = bacc.Bacc("TRN2", target_bir_lowering=False, debug=False,
                   num_devices=N_CORES)
    ft_dram = nc.dram_tensor("ft", [D_FEAT, b_cols], f32, kind="ExternalInput").ap()
    lab_dram = nc.dram_tensor("lab", [b_cols], f32, kind="ExternalInput").ap()
    eye_dram = nc.dram_tensor("eye", [PT, PT], f32, kind="ExternalInput").ap()
    ieye_dram = nc.dram_tensor("ieye", [PT, PT], f32, kind="ExternalInput").ap()
    out_dram = nc.dram_tensor("out", [1, 2], f32, kind="ExternalOutput").ap()

    with tile.TileContext(nc) as tc, ExitStack() as ctx:
        singles = ctx.enter_context(tc.tile_pool(name="singles", bufs=1))
        spsum = ctx.enter_context(tc.tile_pool(name="spsum", bufs=3, space="PSUM"))
        rpsum = ctx.enter_context(tc.tile_pool(name="rpsum", bufs=1, space="PSUM"))
        epool = ctx.enter_context(tc.tile_pool(name="epool", bufs=3))
        t5pool = ctx.enter_context(tc.tile_pool(name="t5pool", bufs=3))
        accpool = ctx.enter_context(tc.tile_pool(name="accpool", bufs=2))
        wpool = ctx.enter_context(tc.tile_pool(name="wpool", bufs=3))
        lrpool = ctx.enter_context(tc.tile_pool(name="lrpool", bufs=2))

        aT = singles.tile([PT, KB, b_cols], f32)       # normalized features^T
        eye = singles.tile([PT, PT], f32)
        ieye = singles.tile([PT, PT], f32)
        nc.sync.dma_start(eye, eye_dram)
        nc.sync.dma_start(ieye, ieye_dram)
        zb = singles.tile([PT, 1], f32)
        nc.vector.memset(zb, 0.0)
        eb = singles.tile([PT, 1], f32)   # Exp bias = -1/T
        nc.vector.memset(eb, -invT)

        # per-row-tile result columns
        denom_all = singles.tile([PT, RT], f32)
        st5_all = singles.tile([PT, RT], f32)
        pc_all = singles.tile([PT, RT], f32)
        A_all = singles.tile([PT, RT], f32)
        PS_all = singles.tile([PT, RT], f32)
        MWE_all = singles.tile([PT, RT], f32)
        sdiag_all = singles.tile([PT, RT], f32)

        # ---- phase 1: norms via PE gram diagonals, then aT = fT * rnorm ----
        with ExitStack() as p1ctx:
            ph1 = p1ctx.enter_context(tc.tile_pool(name="ph1", bufs=1))
            ph1s = p1ctx.enter_context(tc.tile_pool(name="ph1s", bufs=4))
            ph1p = p1ctx.enter_context(tc.tile_pool(name="ph1p", bufs=4, space="PSUM"))
            dramp = p1ctx.enter_context(tc.tile_pool(name="dramp", bufs=1, space="DRAM"))

            fT = ph1.tile([PT, KB, b_cols], f32)
            for k in range(KB):
                nc.sync.dma_start(fT[:, k, :], ft_dram[k * PT:(k + 1) * PT, :])
            rn_dram = dramp.tile([b_cols], f32)
            for u in range(NT_F):
                us = slice(u * PT, (u + 1) * PT)
                dg = ph1s.tile([PT, KB], f32, tag="dg")
                for k in range(KB):
                    gps = ph1p.tile([PT, PT], f32)
                    nc.tensor.matmul(gps, fT[:, k, us], fT[:, k, us],
                                     start=True, stop=True)
                    dsc = ph1s.tile([PT, PT], f32, tag="dsc")
                    nc.vector.scalar_tensor_tensor(dsc, gps, 0.0, eye,
                                                   ALU.bypass, ALU.mult,
                                                   accum_out=dg[:, k:k + 1])
                ss = ph1s.tile([PT, 1], f32, tag="ss")
                nc.vector.tensor_add(ss, dg[:, 0:1], dg[:, 1:2])
                nrm = ph1s.tile([PT, 1], f32, tag="nrm")
                nc.scalar.activation(nrm, ss, AF.Sqrt, bias=zb)
                rn = ph1s.tile([PT, 1], f32, tag="rn")
                nc.vector.reciprocal(rn, nrm)
                nc.sync.dma_start(
                    rn_dram[us].rearrange("(p o) -> p o", o=1), rn)
            rnb = ph1.tile([PT, b_cols], f32)
            nc.sync.dma_start(
                rnb, bass.AP(tensor=rn_dram.tensor, offset=rn_dram.offset,
                             ap=[[0, PT], [1, b_cols]]))
            for k in range(KB):
                for cc in range(NCT):
                    cs = slice(cc * CT, (cc + 1) * CT)
                    nc.vector.tensor_mul(aT[:, k, cs], fT[:, k, cs], rnb[:, cs])

        # ---- phase 2: main sweep -------------------------------------------
        for t in range(RT):
            pieces = all_pieces[t]
            dct, da = (PT * t) // CT, (PT * t) % CT
            l_row = lrpool.tile([PT, 1], f32)
            nc.sync.dma_start(
                l_row, lab_dram[PT * t:PT * (t + 1)].rearrange("(p o) -> p o", o=1))

            acc_e = accpool.tile([PT, NCT + 2], f32, tag="acc_e")
            nc.vector.memset(acc_e, 0.0)
            acc_t5 = accpool.tile([PT, NCT + 2], f32, tag="acc_t5")
            nc.vector.memset(acc_t5, 0.0)
            acc_pc = accpool.tile([PT, npmax], f32, tag="acc_pc")
            acc_A = accpool.tile([PT, npmax], f32, tag="acc_A")
            acc_PS = accpool.tile([PT, npmax], f32, tag="acc_PS")
            acc_MWE = accpool.tile([PT, npmax], f32, tag="acc_MWE")
            for a in (acc_pc, acc_A, acc_PS, acc_MWE):
                nc.vector.memset(a, 0.0)

            for ct in range(NCT):
                ps = spsum.tile([PT, CT], f32)
                for k in range(KB):
                    nc.tensor.matmul(ps, aT[:, k, PT * t:PT * (t + 1)],
                                     aT[:, k, CT * ct:CT * (ct + 1)],
                                     start=(k == 0), stop=(k == KB - 1))
                et = epool.tile([PT, CT], f32)
                t5 = t5pool.tile([PT, CT], f32)
                if ct == dct:
                    # split Exp and the custom pass around the diagonal block
                    if da > 0:
                        nc.scalar.activation(et[:, :da], ps[:, :da], AF.Exp,
                                             bias=eb, scale=invT,
                                             accum_out=acc_e[:, ct:ct + 1])
                        nc.vector._custom_dve(op, out=t5[:, :da], in0=ps[:, :da],
                                              in1=et[:, :da],
                                              s0=1.0 / (1.0 - THR_NEG),
                                              s1=-THR_NEG / (1.0 - THR_NEG),
                                              imm2=1.0,
                                              accum_out=acc_t5[:, ct:ct + 1])
                    dsl = slice(da, da + PT)
                    nc.scalar.activation(et[:, dsl], ps[:, dsl], AF.Exp,
                                         bias=eb, scale=invT)
                    # zero the diagonal into et (scratch out), fused row-sum
                    esc = wpool.tile([PT, PT], f32, tag="esc")
                    nc.vector.scalar_tensor_tensor(esc, et[:, dsl], 0.0, ieye,
                                                   ALU.bypass, ALU.mult,
                                                   accum_out=acc_e[:, NCT:NCT + 1])
                    nc.vector._custom_dve(op, out=t5[:, dsl], in0=ps[:, dsl],
                                          in1=esc,
                                          s0=1.0 / (1.0 - THR_NEG),
                                          s1=-THR_NEG / (1.0 - THR_NEG), imm2=1.0,
                                          accum_out=acc_t5[:, NCT:NCT + 1])
                    if da + PT < CT:
                        psl = slice(da + PT, CT)
                        nc.scalar.activation(et[:, psl], ps[:, psl], AF.Exp,
                                             bias=eb, scale=invT,
                                             accum_out=acc_e[:, NCT + 1:NCT + 2])
                        nc.vector._custom_dve(op, out=t5[:, psl], in0=ps[:, psl],
                                              in1=et[:, psl],
                                              s0=1.0 / (1.0 - THR_NEG),
                                              s1=-THR_NEG / (1.0 - THR_NEG),
                                              imm2=1.0,
                                              accum_out=acc_t5[:, NCT + 1:NCT + 2])
                    # exact diagonal similarity s_ii
                    dsc = wpool.tile([PT, PT], f32, tag="dscr")
                    nc.vector.scalar_tensor_tensor(dsc, ps[:, dsl], 0.0, eye,
                                                   ALU.bypass, ALU.mult,
                                                   accum_out=sdiag_all[:, t:t + 1])
                else:
                    nc.scalar.activation(et, ps, AF.Exp, bias=eb, scale=invT,
                                         accum_out=acc_e[:, ct:ct + 1])
                    nc.vector._custom_dve(op, out=t5, in0=ps, in1=et,
                                          s0=1.0 / (1.0 - THR_NEG),
                                          s1=-THR_NEG / (1.0 - THR_NEG), imm2=1.0,
                                          accum_out=acc_t5[:, ct:ct + 1])
                for pidx, (pct, lo, hi) in enumerate(pieces):
                    if pct != ct:
                        continue
                    w = hi - lo
                    labw = wpool.tile([PT, wmax], f32, tag="labw")
                    nc.sync.dma_start(
                        labw[:, :w],
                        bass.AP(tensor=lab_dram.tensor,
                                offset=lab_dram.offset + ct * CT + lo,
                                ap=[[0, PT], [1, w]]))
                    m_p = wpool.tile([PT, wmax], f32, tag="m_p")
                    nc.vector.tensor_scalar(m_p[:, :w], labw[:, :w], l_row, None,
                                            ALU.is_equal, ALU.add,
                                            accum_out=acc_pc[:, pidx:pidx + 1])
                    mwp = wpool.tile([PT, wmax], f32, tag="mwp")
                    nc.vector._custom_dve(op, out=mwp[:, :w], in0=ps[:, lo:hi],
                                          in1=m_p[:, :w], s0=-1.0, s1=THR_POS,
                                          imm2=1.0,
                                          accum_out=acc_A[:, pidx:pidx + 1])
                    scr = wpool.tile([PT, wmax], f32, tag="scr")
                    nc.vector.scalar_tensor_tensor(scr[:, :w], mwp[:, :w], 0.0,
                                                   ps[:, lo:hi], ALU.bypass,
                                                   ALU.mult,
                                                   accum_out=acc_PS[:, pidx:pidx + 1])
                    scr2 = wpool.tile([PT, wmax], f32, tag="scr2")
                    nc.vector.scalar_tensor_tensor(scr2[:, :w], m_p[:, :w], 0.0,
                                                   t5[:, lo:hi], ALU.bypass,
                                                   ALU.mult,
                                                   accum_out=acc_MWE[:, pidx:pidx + 1])

            nc.vector.reduce_sum(denom_all[:, t:t + 1], acc_e, axis=AX.X)
            nc.vector.reduce_sum(st5_all[:, t:t + 1], acc_t5, axis=AX.X)
            nc.vector.reduce_sum(pc_all[:, t:t + 1], acc_pc, axis=AX.X)
            nc.vector.reduce_sum(A_all[:, t:t + 1], acc_A, axis=AX.X)
            nc.vector.reduce_sum(PS_all[:, t:t + 1], acc_PS, axis=AX.X)
            nc.vector.reduce_sum(MWE_all[:, t:t + 1], acc_MWE, axis=AX.X)

        # ---- phase 3: per-row scalars + final reduction --------------------
        fin = singles.tile
        pcm = fin([PT, RT], f32)      # max(pos_cnt, 1)
        nc.vector.tensor_scalar(pcm, pc_all, 1.0, 1.0, ALU.subtract, ALU.max)
        pinv = fin([PT, RT], f32)
        nc.vector.reciprocal(pinv, pcm)
        ncn = fin([PT, RT], f32)      # neg_cnt = B - pc_raw, clipped at 1
        nc.vector.tensor_scalar(ncn, pc_all, -1.0, float(b_cols), ALU.mult, ALU.add)
        nc.vector.tensor_scalar_max(ncn, ncn, 1.0)
        ninv = fin([PT, RT], f32)
        nc.vector.reciprocal(ninv, ncn)
        logden = fin([PT, RT], f32)
        nc.scalar.activation(logden, denom_all, AF.Ln, bias=zb)
        rden = fin([PT, RT], f32)
        nc.vector.reciprocal(rden, denom_all)
        Ac = fin([PT, RT], f32)
        nc.vector.tensor_scalar_sub(Ac, A_all, 1.0)
        PSc = fin([PT, RT], f32)
        nc.vector.tensor_sub(PSc, PS_all, sdiag_all)
        t1 = fin([PT, RT], f32)
        nc.vector.tensor_sub(t1, PSc, Ac)
        t2 = fin([PT, RT], f32)
        nc.vector.tensor_mul(t2, logden, Ac)
        possum = fin([PT, RT], f32)
        nc.vector.scalar_tensor_tensor(possum, t1, invT, t2, ALU.mult, ALU.subtract)
        resv = fin([PT, 2], f32)
        junk1 = fin([PT, RT], f32)
        nc.vector.scalar_tensor_tensor(junk1, possum, 0.0, pinv, ALU.bypass,
                                       ALU.mult, accum_out=resv[:, 0:1])
        E = fin([PT, RT], f32)
        nc.vector.tensor_sub(E, st5_all, MWE_all)
        t4 = fin([PT, RT], f32)
        nc.vector.tensor_mul(t4, E, rden)
        junk2 = fin([PT, RT], f32)
        nc.vector.scalar_tensor_tensor(junk2, t4, 0.0, ninv, ALU.bypass,
                                       ALU.mult, accum_out=resv[:, 1:2])
        ones = fin([PT, 1], f32)
        nc.vector.memset(ones, 1.0)
        psr = rpsum.tile([1, 2], f32)
        nc.tensor.matmul(psr, ones, resv, start=True, stop=True)
        outs = fin([1, 2], f32)
        nc.scalar.copy(outs, psr)
        nc.sync.dma_start(out_dram, outs)

    nc.compile()
    return nc


# ---- host orchestration ----------------------------------------------------
def _prep(features, labels, n_cores):
    features = np.ascontiguousarray(np.asarray(features, dtype=np.float32))
    labels = np.asarray(labels).astype(np.int64)
    b = features.shape[0]
    order = np.argsort(labels, kind="stable")
    f_s = features[order]
    l_s = labels[order].astype(np.float32)
    counts = np.bincount(labels)
    pad = int(max(counts.max() - 1, 0))
    r = b // n_cores
    eye = np.eye(PT, dtype=np.float32)
    ieye = (1.0 - eye).astype(np.float32)
    in_maps = []
    for c in range(n_cores):
        sh = c * r
        in_maps.append({
            "ft": np.ascontiguousarray(np.roll(f_s, -sh, axis=0).T),
            "lab": np.ascontiguousarray(np.roll(l_s, -sh)),
            "eye": eye,
            "ieye": ieye,
        })
    return in_maps, pad, r, b


def _combine(results, b):
    p = sum(float(r["out"][0, 0]) for r in results)
    n = sum(float(r["out"][0, 1]) for r in results)
    loss = -p / b + NEG_LOSS_W * (n / b)
    return np.float32(loss)


def kernel(features, labels):
    global LAST_RESULTS
    from concourse import bass_utils

    in_maps, pad, r, b = _prep(features, labels, N_CORES)
    key = (b, r, pad)
    if key not in _prog_cache:
        _prog_cache[key] = _build(b, r, pad)
    nc = _prog_cache[key]
    res = bass_utils.run_bass_kernel_spmd(nc, in_maps, core_ids=list(range(N_CORES)))
    LAST_RESULTS = res
    return _combine(res.results, b)


def kernel_sim(features, labels, n_cores=N_CORES):
    """CoreSim-backed variant for correctness testing (no hardware)."""
    from concourse.bass_interp import CoreSim

    in_maps, pad, r, b = _prep(features, labels, n_cores)
    nc = _build_for(b, r, pad, n_cores)
    results = []
    for c in range(n_cores):
        sim = CoreSim(nc, trace=False)
        for name, arr in in_maps[c].items():
            sim.tensor(name)[:] = arr
        sim.simulate(check_with_hw=False)
        results.append({"out": np.array(sim.tensor("out"))})
    return _combine(results, b)


def _build_for(b, r, pad, n_cores):
    key = (b, r, pad)
    if key not in _prog_cache:
        _prog_cache[key] = _build(b, r, pad)
    return _prog_cache[key]
